# revision 15
# baseline (speedup 1.0000x reference)
"""Trainium2 Bass kernel for nn_MentionScore (ragged span mention scoring).

Self-contained: takes FULL inputs as numpy arrays, shards 16 docs across
8 NeuronCores (2 docs/core), runs a Bass/Tile program per core, returns
(span_emb [16,2000,3072] f32, mention_scores [16,2000,1] f32).

Per-doc device pipeline:
  A) load x, cast bf16, transpose-gather X^T, attention MLP (bf16) -> a[t];
     w = a*x (rounded f32r) in place; block-total accumulation on PE
  B) cumsum via triangular matmul (f32r) + block offsets (bf16 hi/lo split),
     write csum to DRAM (f32) and keep a bf16 copy in SBUF
  C) per 256-span group: f32 row gathers of x/csum -> span_emb scatter-out;
     bf16 transposed gathers -> span MLP -> mention scores
Ragged handling: per-group runtime counts (SWDGE num_idxs_reg) skip masked
spans entirely; outputs are zero-donated so skipped rows stay zero.
"""
import numpy as np
import ml_dtypes

import concourse.bass as bass
import concourse.tile as tile
from concourse import bacc, mybir
from concourse import bass_utils

F32 = mybir.dt.float32
F32R = mybir.dt.float32r
BF16 = mybir.dt.bfloat16
I16 = mybir.dt.int16
I32 = mybir.dt.int32
AF = mybir.ActivationFunctionType
OP = mybir.AluOpType

B, T, E, S, HID = 16, 2048, 1024, 2000, 150
GI = 3 * E
NCORES = 8
D = B // NCORES          # docs per core = 2
P = 128
NB = T // P              # 16 token blocks
GS = 256                 # spans per group
NG = 2048 // GS          # 8 groups
SPAD = NG * GS           # 2048
EC = E // P              # 8 feature chunks
NT = T // 512            # 4 token Ntiles
H2 = HID - P             # 22

_PROGRAM_CACHE = {}


def _wrap_idx(arr):
    """[n] -> [128, n//16] int16 wrapped (idx i at [i%16, i//16]), replicated x8."""
    n = len(arr)
    a = np.asarray(arr, np.int16).reshape(n // 16, 16).T.copy()
    return np.tile(a, (8, 1))


def _build_program(trace_scopes=False):
    nc = bacc.Bacc("TRN2", target_bir_lowering=False, debug=False)

    xs = [nc.dram_tensor(f"x{d}", [T, E], F32, kind="ExternalInput").ap() for d in range(D)]
    w_in = {}
    for nm, shp in [("Wa1", [E, HID]), ("Wa2", [HID, HID]), ("Wa3", [HID, 1]),
                    ("Ws1", [GI, HID]), ("Ws2", [HID, HID]), ("Ws3", [HID, 1]),
                    ("ba1", [HID, 1]), ("ba2", [HID, 1]), ("bs1", [HID, 1]), ("bs2", [HID, 1]),
                    ("ba3b", [P, 1]), ("bs3b", [P, 1])]:
        w_in[nm] = nc.dram_tensor(nm, shp, F32, kind="ExternalInput").ap()
    c_U = nc.dram_tensor("U128", [P, P], F32R, kind="ExternalInput").ap()
    c_oh = nc.dram_tensor("onehot64", [P, NB * 64], F32R, kind="ExternalInput").ap()
    c_sm = nc.dram_tensor("stepmask64", [64, NB * P], BF16, kind="ExternalInput").ap()
    c_iota = nc.dram_tensor("ixiota", [P, T // 16], I16, kind="ExternalInput").ap()
    ix_in = {(nm, d): nc.dram_tensor(f"ix_{nm}{d}", [P, NG * (GS // 16)], I16,
                                     kind="ExternalInput").ap()
             for nm in ["gs", "ge", "c1", "cs0", "cs1", "row"] for d in range(D)}
    cnt_in = nc.dram_tensor("cnt", [1, D * NG], I32, kind="ExternalInput").ap()
    scmask_in = [nc.dram_tensor(f"scmask{d}", [P, SPAD // P], I32, kind="ExternalInput").ap()
                 for d in range(D)]
    se_out = [nc.dram_tensor(f"se{d}", [SPAD + 1, GI], F32, kind="ExternalOutput").ap()
              for d in range(D)]
    sc_out = [nc.dram_tensor(f"sc{d}", [P, SPAD // P], F32, kind="ExternalOutput").ap()
              for d in range(D)]

    with tile.TileContext(nc) as tc:
        with tc.tile_pool(name="wpool", bufs=1) as wp, \
             tc.tile_pool(name="dram", bufs=1, space="DRAM") as dr:

            # ------------- weights/consts to SBUF (once, bf16 cast in DMA) -------------
            Wa1_16 = wp.tile([P, EC, HID], BF16)
            nc.gpsimd.dma_start(Wa1_16[:], w_in["Wa1"].rearrange("(k p) h -> p k h", p=P))

            W1_16 = wp.tile([P, 3 * EC, HID], BF16)
            nc.gpsimd.dma_start(W1_16[:], w_in["Ws1"].rearrange("(k p) h -> p k h", p=P))
            W1n_16 = wp.tile([P, EC, HID], BF16)
            nc.vector.tensor_scalar_mul(W1n_16[:], W1_16[:, 2 * EC:3 * EC, :], -1.0)

            def load_pair(name, cols):
                ba = wp.tile([P, cols], BF16, tag=f"{name}ba")
                nc.gpsimd.dma_start(ba[:], w_in[name][0:P, :])
                bb = wp.tile([H2, cols], BF16, tag=f"{name}bb")
                nc.gpsimd.dma_start(bb[:], w_in[name][P:HID, :])
                return ba, bb

            Wa2a, Wa2b = load_pair("Wa2", HID)
            Wa3a, Wa3b = load_pair("Wa3", 1)
            Ws2a, Ws2b = load_pair("Ws2", HID)
            Ws3a, Ws3b = load_pair("Ws3", 1)

            bias_t = {}
            for nm in ["ba1", "ba2", "bs1", "bs2"]:
                ta = wp.tile([P, 1], F32, tag=f"{nm}a")
                nc.sync.dma_start(ta[:], w_in[nm][0:P, :])
                tb = wp.tile([H2, 1], F32, tag=f"{nm}b")
                nc.sync.dma_start(tb[:], w_in[nm][P:HID, :])
                bias_t[nm] = (ta, tb)
            ba3b_t = wp.tile([P, 1], F32)
            nc.sync.dma_start(ba3b_t[:], w_in["ba3b"])
            bs3b_t = wp.tile([P, 1], F32)
            nc.sync.dma_start(bs3b_t[:], w_in["bs3b"])

            U128_t = wp.tile([P, P], F32R)
            nc.sync.dma_start(U128_t[:], c_U)
            oh64_t = wp.tile([P, NB * 64], F32R)
            nc.sync.dma_start(oh64_t[:], c_oh)
            sm64_t = wp.tile([64, NB * P], BF16)
            nc.sync.dma_start(sm64_t[:], c_sm)
            ixiota_t = wp.tile([P, T // 16], I16)
            nc.sync.dma_start(ixiota_t[:], c_iota)
            zrow = wp.tile([1, E], F32)
            nc.vector.memset(zrow[:], 0.0)
            cnt_t = wp.tile([1, D * NG], I32)
            nc.sync.dma_start(cnt_t[:], cnt_in)

            csum32 = [dr.tile([T + 1, E], F32, tag=f"csum32_{d}", name=f"csum32_{d}")
                      for d in range(D)]
            creg = nc.gpsimd.alloc_register()

            # ---------------- per-doc pipeline ----------------
            for d in range(D):
                with tc.tile_pool(name=f"xc{d}", bufs=1) as xc:
                    xb16 = xc.tile([P, NB, E], BF16)
                    cs16 = xc.tile([P, NB + 1, E], BF16)
                    nc.vector.memset(cs16[:, 0, :], 0.0)

                    with tc.tile_pool(name=f"ab{d}", bufs=1) as ab, \
                         tc.tile_pool(name=f"ps_ab{d}", bufs=1, space="PSUM") as psA, \
                         tc.tile_pool(name=f"cs_sb{d}", bufs=2) as csp, \
                         tc.tile_pool(name=f"ps_c{d}", bufs=2, space="PSUM") as psC, \
                         tc.tile_pool(name=f"xt{d}", bufs=2) as xtp:

                        wbuf = ab.tile([P, NB // 2, E], F32R)   # one round (8 blocks)
                        a_sb = ab.tile([P, NB], F32)
                        TOT_ps = psA.tile([64, E], F32, tag="totps")
                        hilo = ab.tile([64, E], BF16)
                        TOT2 = ab.tile([64, E], F32)
                        tmpf = ab.tile([64, E], F32)
                        nc.sync.dma_start(csum32[d][0:1, :], zrow[:])

                        for r in range(2):
                            # ---- pass A: per 512-token slab ----
                            for j in (2 * r, 2 * r + 1):
                                lb = 4 * (j - 2 * r)   # local block base in wbuf
                                xload = xtp.tile([P, 4, E], F32, tag="xload")
                                nc.sync.dma_start(
                                    xload[:],
                                    xs[d][512 * j:512 * (j + 1), :]
                                    .rearrange("(r p) e -> p r e", p=P))
                                nc.scalar.activation(xb16[:, 4 * j:4 * (j + 1), :],
                                                     xload[:], AF.Identity)
                                xt = xtp.tile([P, EC, 512], BF16, tag="xt")
                                nc.gpsimd.dma_gather(
                                    xt[:], xb16[:], ixiota_t[:, 32 * j:32 * (j + 1)], 512,
                                    512, E, transpose=True, sbuf_tokens_per_rank=P,
                                    sbuf_free_dim_per_rank=2 * E)
                                a1 = psA.tile([P, 512], F32, tag="apsum")
                                a1b = psA.tile([H2, 512], F32, tag="apsumb")
                                for k in range(EC):
                                    nc.tensor.matmul(a1[:], Wa1_16[:, k, 0:P], xt[:, k, :],
                                                     start=(k == 0), stop=(k == EC - 1))
                                for k in range(EC):
                                    nc.tensor.matmul(a1b[:], Wa1_16[:, k, P:HID], xt[:, k, :],
                                                     start=(k == 0), stop=(k == EC - 1))
                                h1a = xtp.tile([P, 512], BF16, tag="h1a")
                                h1b = xtp.tile([H2, 512], BF16, tag="h1b")
                                nc.scalar.activation(h1a[:], a1[:], AF.Relu,
                                                     bias=bias_t["ba1"][0][:])
                                nc.scalar.activation(h1b[:], a1b[:], AF.Relu,
                                                     bias=bias_t["ba1"][1][:])
                                a2 = psA.tile([P, 512], F32, tag="apsum")
                                a2b = psA.tile([H2, 512], F32, tag="apsumb")
                                nc.tensor.matmul(a2[:], Wa2a[:, 0:P], h1a[:],
                                                 start=True, stop=False)
                                nc.tensor.matmul(a2[:], Wa2b[:, 0:P], h1b[:],
                                                 start=False, stop=True)
                                nc.tensor.matmul(a2b[:], Wa2a[:, P:HID], h1a[:],
                                                 start=True, stop=False)
                                nc.tensor.matmul(a2b[:], Wa2b[:, P:HID], h1b[:],
                                                 start=False, stop=True)
                                h2a = xtp.tile([P, 512], BF16, tag="h2a")
                                h2b = xtp.tile([H2, 512], BF16, tag="h2b")
                                nc.scalar.activation(h2a[:], a2[:], AF.Relu,
                                                     bias=bias_t["ba2"][0][:])
                                nc.scalar.activation(h2b[:], a2b[:], AF.Relu,
                                                     bias=bias_t["ba2"][1][:])
                                aps_full = psA.tile([P, 512], F32, tag="apsum", name="aps")
                                aps = aps_full[:, 0:4]
                                for jj in range(4):
                                    nc.tensor.matmul(aps[:, jj:jj + 1],
                                                     h2a[:, P * jj:P * (jj + 1)], Wa3a[:],
                                                     start=True, stop=False)
                                    nc.tensor.matmul(aps[:, jj:jj + 1],
                                                     h2b[:, P * jj:P * (jj + 1)], Wa3b[:],
                                                     start=False, stop=True)
                                nc.vector.tensor_scalar(a_sb[:, 4 * j:4 * (j + 1)], aps[:],
                                                        ba3b_t[:], None, OP.add)
                                for bb in range(4 * j, 4 * (j + 1)):
                                    lbb = bb - 8 * r
                                    nc.vector.tensor_scalar_mul(
                                        wbuf[:, lbb, :], xload[:, bb - 4 * j, :],
                                        a_sb[:, bb:bb + 1])
                                    for h in range(2):
                                        nc.tensor.matmul(
                                            TOT_ps[:, 512 * h:512 * (h + 1)],
                                            oh64_t[:, 64 * bb:64 * (bb + 1)],
                                            wbuf[:, lbb, 512 * h:512 * (h + 1)],
                                            start=(bb == 0), stop=(bb % 8 == 7),
                                            skip_group_check=True)

                            # ---- block offsets hi/lo split (rows beyond round are 0) ----
                            nc.vector.tensor_copy(TOT2[:], TOT_ps[:])
                            nc.scalar.activation(hilo[:], TOT2[:], AF.Identity)
                            nc.vector.tensor_copy(tmpf[32:48, :], hilo[32:48, :])
                            nc.vector.tensor_tensor(tmpf[32:48, :], TOT2[32:48, :],
                                                    tmpf[32:48, :], op=OP.subtract)
                            nc.vector.tensor_copy(hilo[32:48, :], tmpf[32:48, :])

                            # ---- pass B: cumsum for this round's blocks ----
                            for bb in range(8 * r, 8 * r + 8):
                                lbb = bb - 8 * r
                                cps = psC.tile([P, E], F32, tag="cps")
                                for h in range(2):
                                    only = (bb == 0)
                                    nc.tensor.matmul(
                                        cps[:, 512 * h:512 * (h + 1)], U128_t[:],
                                        wbuf[:, lbb, 512 * h:512 * (h + 1)],
                                        start=True, stop=only)
                                    if not only:
                                        nc.tensor.matmul(
                                            cps[:, 512 * h:512 * (h + 1)],
                                            sm64_t[:, P * bb:P * (bb + 1)],
                                            hilo[:, 512 * h:512 * (h + 1)],
                                            start=False, stop=True)
                                csb = csp.tile([P, E], F32, tag="csb")
                                nc.vector.tensor_copy(csb[:], cps[:])
                                nc.scalar.activation(cs16[:, bb + 1, :], cps[:], AF.Identity)
                                nc.sync.dma_start(
                                    csum32[d][1 + P * bb:1 + P * (bb + 1), :], csb[:])

                    # ---- pass C: span groups ----
                    with tc.tile_pool(name=f"span{d}", bufs=2) as sp, \
                         tc.tile_pool(name=f"span1{d}", bufs=2) as sp1, \
                         tc.tile_pool(name=f"ix{d}", bufs=1) as ixp, \
                         tc.tile_pool(name=f"ps_s{d}", bufs=1, space="PSUM") as psS:

                        ix_t = {}
                        for nm in ["gs", "ge", "c1", "cs0", "cs1", "row"]:
                            t = ixp.tile([P, NG * (GS // 16)], I16, tag=f"ix{nm}")
                            nc.sync.dma_start(t[:], ix_in[(nm, d)])
                            ix_t[nm] = t
                        scps = psS.tile([P, SPAD // P], F32, tag="scps")

                        for g in range(NG):
                            gsl = slice((GS // 16) * g, (GS // 16) * (g + 1))
                            nc.gpsimd.load(creg, cnt_t[0:1, d * NG + g:d * NG + g + 1])

                            gs32 = sp.tile([P, GS // P, E], F32, tag="gs32")
                            ge32 = sp.tile([P, GS // P, E], F32, tag="ge32")
                            c0g = sp.tile([P, GS // P, E], F32, tag="c0g")
                            c1g = sp.tile([P, GS // P, E], F32, tag="c1g")
                            nc.gpsimd.dma_gather(gs32[:], xs[d], ix_t["gs"][:, gsl],
                                                 GS, creg, E)
                            nc.gpsimd.dma_gather(ge32[:], xs[d], ix_t["ge"][:, gsl],
                                                 GS, creg, E)
                            nc.gpsimd.dma_gather(c0g[:], csum32[d][:], ix_t["gs"][:, gsl],
                                                 GS, creg, E)
                            nc.gpsimd.dma_gather(c1g[:], csum32[d][:], ix_t["c1"][:, gsl],
                                                 GS, creg, E)
                            nc.vector.tensor_tensor(c1g[:], c1g[:], c0g[:], op=OP.subtract)
                            nc.gpsimd.dma_scatter_add(se_out[d][:, 0:E], gs32[:],
                                                      ix_t["row"][:, gsl], GS, creg, E,
                                                      elem_step=GI)
                            nc.gpsimd.dma_scatter_add(se_out[d][:, E:2 * E], ge32[:],
                                                      ix_t["row"][:, gsl], GS, creg, E,
                                                      elem_step=GI)
                            nc.gpsimd.dma_scatter_add(se_out[d][:, 2 * E:3 * E], c1g[:],
                                                      ix_t["row"][:, gsl], GS, creg, E,
                                                      elem_step=GI)

                            tg = {}
                            for nm, src, ixnm in [("gs", xb16, "gs"), ("ge", xb16, "ge"),
                                                  ("c0", cs16, "cs0"), ("c1", cs16, "cs1")]:
                                tt = sp1.tile([P, EC, GS], BF16, tag=f"tg_{nm}")
                                nc.gpsimd.dma_gather(
                                    tt[:], src[:], ix_t[ixnm][:, gsl], GS, creg, E,
                                    transpose=True, sbuf_tokens_per_rank=P,
                                    sbuf_free_dim_per_rank=2 * E)
                                tg[nm] = tt

                            h1 = psS.tile([P, GS], F32, tag="h1")
                            h1b = psS.tile([H2, GS], F32, tag="h1b")
                            srcs = [("gs", 0), ("ge", EC), ("c1", 2 * EC), ("c0", None)]
                            n_mm = 4 * EC
                            i_mm = 0
                            for nm, koff in srcs:
                                for k in range(EC):
                                    w_ap = (W1n_16[:, k, :] if koff is None
                                            else W1_16[:, koff + k, :])
                                    st, en = (i_mm == 0), (i_mm == n_mm - 1)
                                    nc.tensor.matmul(h1[:], w_ap[:, 0:P], tg[nm][:, k, :],
                                                     start=st, stop=en)
                                    nc.tensor.matmul(h1b[:], w_ap[:, P:HID], tg[nm][:, k, :],
                                                     start=st, stop=en)
                                    i_mm += 1
                            s1a = sp1.tile([P, GS], BF16, tag="s1a")
                            s1b = sp1.tile([H2, GS], BF16, tag="s1b")
                            nc.scalar.activation(s1a[:], h1[:], AF.Relu,
                                                 bias=bias_t["bs1"][0][:])
                            nc.scalar.activation(s1b[:], h1b[:], AF.Relu,
                                                 bias=bias_t["bs1"][1][:])
                            h2 = psS.tile([P, GS], F32, tag="h2")
                            h2b = psS.tile([H2, GS], F32, tag="h2b")
                            nc.tensor.matmul(h2[:], Ws2a[:, 0:P], s1a[:], start=True, stop=False)
                            nc.tensor.matmul(h2[:], Ws2b[:, 0:P], s1b[:], start=False, stop=True)
                            nc.tensor.matmul(h2b[:], Ws2a[:, P:HID], s1a[:],
                                             start=True, stop=False)
                            nc.tensor.matmul(h2b[:], Ws2b[:, P:HID], s1b[:],
                                             start=False, stop=True)
                            s2a = sp1.tile([P, GS], BF16, tag="s2a")
                            s2b = sp1.tile([H2, GS], BF16, tag="s2b")
                            nc.scalar.activation(s2a[:], h2[:], AF.Relu,
                                                 bias=bias_t["bs2"][0][:])
                            nc.scalar.activation(s2b[:], h2b[:], AF.Relu,
                                                 bias=bias_t["bs2"][1][:])
                            for c in range(GS // P):
                                col = g * (GS // P) + c
                                nc.tensor.matmul(scps[:, col:col + 1],
                                                 s2a[:, P * c:P * (c + 1)], Ws3a[:],
                                                 start=True, stop=False)
                                nc.tensor.matmul(scps[:, col:col + 1],
                                                 s2b[:, P * c:P * (c + 1)], Ws3b[:],
                                                 start=False, stop=True)

                        scmask_t = ixp.tile([P, SPAD // P], I32, tag="scmask")
                        nc.sync.dma_start(scmask_t[:], scmask_in[d])
                        sc_f = ixp.tile([P, SPAD // P], F32, tag="scf")
                        sc_z = ixp.tile([P, SPAD // P], F32, tag="scz")
                        sc_m = ixp.tile([P, SPAD // P], F32, tag="scm2")
                        nc.vector.memset(sc_z[:], 0.0)
                        nc.vector.tensor_scalar(sc_f[:], scps[:], bs3b_t[:], None, OP.add)
                        nc.vector.select(sc_m[:], scmask_t[:], sc_f[:], sc_z[:])
                        nc.sync.dma_start(sc_out[d][:], sc_m[:])

    nc.compile()
    return nc


def _get_program():
    if "nc" not in _PROGRAM_CACHE:
        _PROGRAM_CACHE["nc"] = _build_program()
    return _PROGRAM_CACHE["nc"]


def _host_prep(inputs):
    """Build the 8 per-core input maps."""
    x = np.ascontiguousarray(np.asarray(inputs["batch_embeds"], dtype=np.float32))
    starts = np.asarray(inputs["starts"]).astype(np.int64)
    lengths = np.asarray(inputs["lengths"]).astype(np.int64)
    num_spans = np.asarray(inputs["num_spans"]).astype(np.int64)
    ends = starts + lengths

    wmap = {}
    for nm in ["Wa1", "Wa2", "Wa3", "Ws1", "Ws2", "Ws3"]:
        wmap[nm] = np.ascontiguousarray(np.asarray(inputs[nm], dtype=np.float32))
    for nm in ["ba1", "ba2", "bs1", "bs2"]:
        wmap[nm] = np.asarray(inputs[nm], dtype=np.float32).reshape(HID, 1)
    wmap["ba3b"] = np.broadcast_to(
        np.asarray(inputs["ba3"], np.float32).reshape(1, 1), (P, 1)).copy()
    wmap["bs3b"] = np.broadcast_to(
        np.asarray(inputs["bs3"], np.float32).reshape(1, 1), (P, 1)).copy()

    U = np.triu(np.ones((P, P), np.float32))
    onehot = np.zeros((P, NB * 64), np.float32)
    for bb in range(NB):
        onehot[:, 64 * bb + bb] = 1.0
        onehot[:, 64 * bb + 32 + bb] = 1.0
    stepmask = np.zeros((64, NB * P), np.float32)
    for bb in range(NB):
        for k in range(64):
            if (k < 16 and k < bb) or (32 <= k < 48 and k - 32 < bb):
                stepmask[k, P * bb:P * (bb + 1)] = 1.0
    ixiota = _wrap_idx(np.arange(T))

    in_maps = []
    for c in range(NCORES):
        m = dict(wmap)
        m["U128"] = U
        m["onehot64"] = onehot
        m["stepmask64"] = stepmask.astype(ml_dtypes.bfloat16)
        m["ixiota"] = ixiota
        cnts = np.zeros((1, D * NG), np.int32)
        for d in range(D):
            doc = c * D + d
            ns = int(num_spans[doc])
            m[f"x{d}"] = x[doc]
            st, en = starts[doc], ends[doc]
            for nm, vals, dummy0 in [
                ("gs", st, 0), ("ge", en, 0), ("c1", en + 1, 0),
                ("cs0", st + P - 1, P - 1), ("cs1", en + P, P),
                ("row", np.arange(S), SPAD),
            ]:
                a = np.full(SPAD, -1, np.int64)
                a[:min(ns, S)] = vals[:min(ns, S)]
                wrapped = np.empty((P, NG * (GS // 16)), np.int16)
                for g in range(NG):
                    seg = a[GS * g:GS * (g + 1)].copy()
                    cnt = max(0, min(ns - GS * g, GS))
                    if cnt == 0:
                        seg[0] = dummy0
                        cnt = 1
                    wrapped[:, (GS // 16) * g:(GS // 16) * (g + 1)] = _wrap_idx(seg)
                    if nm == "gs":
                        cnts[0, d * NG + g] = cnt
                m[f"ix_{nm}{d}"] = wrapped
            scm = np.zeros(SPAD, np.int32)
            scm[:ns] = 1
            m[f"scmask{d}"] = scm.reshape(SPAD // P, P).T.copy()
        m["cnt"] = cnts
        in_maps.append(m)
    return in_maps


LAST_RESULTS = None


def kernel(**inputs):
    global LAST_RESULTS
    nc = _get_program()
    in_maps = _host_prep(inputs)
    res = bass_utils.run_bass_kernel_spmd(nc, in_maps, core_ids=list(range(NCORES)))
    LAST_RESULTS = res
    span_emb = np.empty((B, S, GI), np.float32)
    scores = np.empty((B, S, 1), np.float32)
    for c in range(NCORES):
        r = res.results[c]
        for d in range(D):
            doc = c * D + d
            span_emb[doc] = r[f"se{d}"][:S, :]
            sc = r[f"sc{d}"].T.reshape(SPAD)[:S]
            scores[doc] = sc[:, None]
    return span_emb, scores


if __name__ == "__main__":
    import time
    ref = np.load("/root/problem/work/ref_cache.npz")
    inputs = {k: ref[k] for k in ref.files if k not in ("span_emb", "mention_scores")}
    t0 = time.time()
    se, ms = kernel(**inputs)
    print("run time", time.time() - t0)
    np.savez("/root/problem/work/dbg_out.npz", se=se, ms=ms)
    for nm, got, want in [("span_emb", se, ref["span_emb"]),
                          ("scores", ms, ref["mention_scores"])]:
        rel = np.linalg.norm((got - want).ravel()) / np.linalg.norm(want.ravel())
        print(f"{nm}: rel={rel:.3e} maxabs={np.abs(got - want).max():.3e}")


# revision 16
# speedup vs baseline: 1.2824x; 1.2824x over previous
"""Trainium2 Bass kernel for nn_MentionScore (ragged span mention scoring).

Self-contained: takes FULL inputs as numpy arrays, shards 16 docs across
8 NeuronCores (2 docs/core), runs a Bass/Tile program per core, returns
(span_emb [16,2000,3072] f32, mention_scores [16,2000,1] f32).

v2 design (prefix-sum/projection form):
  A) per 512-token slab: load x, cast bf16 -> DRAM, HWDGE-transpose to X^T;
     attention MLP (bf16) -> a[t] (token-major and row forms);
     projections Pa/Pb/Pc = X @ Ws1-parts (feature-major bf16);
     w = a*x (f32r); block-total accumulation on PE
  B) cumsum via triangular matmul (f32r) + block offsets (bf16 hi/lo),
     csum -> DRAM f32   [two 8-block rounds to halve the w buffer]
  S) y = a.Pc, Qi = scan(y); R = Pa - Q_excl, S = Pb + Q_incl; PE-transpose
     to token-major bf16 tables
  C) span groups: f32 row gathers (x,csum) -> assembled span_emb rows ->
     single scatter per group; bf16 gathers of R[s], S[e] -> h1 = R+S ->
     span MLP L2/L3 -> scores
Ragged: runtime per-group counts (num_idxs_reg) skip masked spans; outputs
are zero-donated so skipped rows stay zero.
"""
import numpy as np
import ml_dtypes

import concourse.bass as bass
import concourse.tile as tile
from concourse import bacc, mybir
from concourse import bass_utils

F32 = mybir.dt.float32
F32R = mybir.dt.float32r
BF16 = mybir.dt.bfloat16
I16 = mybir.dt.int16
I32 = mybir.dt.int32
AF = mybir.ActivationFunctionType
OP = mybir.AluOpType

B, T, E, S, HID = 16, 2048, 1024, 2000, 150
GI = 3 * E
NCORES = 8
D = B // NCORES          # docs per core
P = 128
NB = T // P              # 16 token blocks
GSF = 256                # f32-path spans per group
NGF = 2048 // GSF        # 8
GSB = 512                # bf16-path spans per group
NGB = 2048 // GSB        # 4
SPAD = 2048
EC = E // P              # 8 feature chunks
NT = T // 512            # 4 token Ntiles
H2 = HID - P             # 22
RP = 256                 # padded row size of R/S tables (bf16)

_PROGRAM_CACHE = {}


def _wrap_idx(arr):
    """[n] -> [128, n//16] int16 wrapped (idx i at [i%16, i//16]), replicated x8."""
    n = len(arr)
    a = np.asarray(arr, np.int16).reshape(n // 16, 16).T.copy()
    return np.tile(a, (8, 1))


def _build_program():
    nc = bacc.Bacc("TRN2", target_bir_lowering=False, debug=False)

    xs = [nc.dram_tensor(f"x{d}", [T, E], F32, kind="ExternalInput").ap() for d in range(D)]
    w_in = {}
    for nm, shp in [("Wa1", [E, HID]), ("Wa2", [HID, HID]), ("Wa3", [HID, 1]),
                    ("Ws1", [GI, HID]), ("Ws2", [HID, HID]), ("Ws3", [HID, 1]),
                    ("ba1", [HID, 1]), ("ba2", [HID, 1]), ("bs1", [HID, 1]), ("bs2", [HID, 1]),
                    ("ba3b", [P, 1]), ("bs3b", [P, 1])]:
        w_in[nm] = nc.dram_tensor(nm, shp, F32, kind="ExternalInput").ap()
    c_U = nc.dram_tensor("U128", [P, P], F32R, kind="ExternalInput").ap()
    c_oh = nc.dram_tensor("onehot64", [P, NB * 64], F32R, kind="ExternalInput").ap()
    c_sm = nc.dram_tensor("stepmask64", [64, NB * P], BF16, kind="ExternalInput").ap()
    c_id = nc.dram_tensor("ident", [P, P], BF16, kind="ExternalInput").ap()
    c_ones = nc.dram_tensor("ones1", [1, P], BF16, kind="ExternalInput").ap()
    ix_in = {(nm, d): nc.dram_tensor(f"ix_{nm}{d}", [P, SPAD // 16], I16,
                                     kind="ExternalInput").ap()
             for nm in ["gsF", "geF", "c1F", "row", "gsB", "geB"] for d in range(D)}
    cntF_in = nc.dram_tensor("cntF", [1, D * NGF], I32, kind="ExternalInput").ap()
    cntB_in = nc.dram_tensor("cntB", [1, D * NGB], I32, kind="ExternalInput").ap()
    scmask_in = [nc.dram_tensor(f"scmask{d}", [P, SPAD // P], I32, kind="ExternalInput").ap()
                 for d in range(D)]
    se_out = [nc.dram_tensor(f"se{d}", [SPAD + 1, GI], F32, kind="ExternalOutput").ap()
              for d in range(D)]
    sc_out = [nc.dram_tensor(f"sc{d}", [P, SPAD // P], F32, kind="ExternalOutput").ap()
              for d in range(D)]

    with tile.TileContext(nc) as tc:
        with tc.tile_pool(name="wpool", bufs=1) as wp, \
             tc.tile_pool(name="dram", bufs=1, space="DRAM") as dr:

            # ------------- weights/consts (bf16 cast during DMA) -------------
            Wa1_16 = wp.tile([P, EC, HID], BF16)
            nc.gpsimd.dma_start(Wa1_16[:], w_in["Wa1"].rearrange("(k p) h -> p k h", p=P))
            W1_16 = wp.tile([P, 3 * EC, HID], BF16)
            nc.gpsimd.dma_start(W1_16[:], w_in["Ws1"].rearrange("(k p) h -> p k h", p=P))

            def load_pair(name, cols):
                ba = wp.tile([P, cols], BF16, tag=f"{name}ba")
                nc.gpsimd.dma_start(ba[:], w_in[name][0:P, :])
                bb = wp.tile([H2, cols], BF16, tag=f"{name}bb")
                nc.gpsimd.dma_start(bb[:], w_in[name][P:HID, :])
                return ba, bb

            Wa2a, Wa2b = load_pair("Wa2", HID)
            Wa3a, Wa3b = load_pair("Wa3", 1)
            Ws2a, Ws2b = load_pair("Ws2", HID)
            Ws3a, Ws3b = load_pair("Ws3", 1)

            bias_t = {}
            for nm in ["ba1", "ba2", "bs1", "bs2"]:
                ta = wp.tile([P, 1], F32, tag=f"{nm}a")
                nc.sync.dma_start(ta[:], w_in[nm][0:P, :])
                tb = wp.tile([H2, 1], F32, tag=f"{nm}b")
                nc.sync.dma_start(tb[:], w_in[nm][P:HID, :])
                bias_t[nm] = (ta, tb)
            ba3b_t = wp.tile([P, 1], F32)
            nc.sync.dma_start(ba3b_t[:], w_in["ba3b"])
            bs3b_t = wp.tile([P, 1], F32)
            nc.sync.dma_start(bs3b_t[:], w_in["bs3b"])

            U128_t = wp.tile([P, P], F32R)
            nc.sync.dma_start(U128_t[:], c_U)
            oh64_t = wp.tile([P, NB * 64], F32R)
            nc.sync.dma_start(oh64_t[:], c_oh)
            sm64_t = wp.tile([64, NB * P], BF16)
            nc.sync.dma_start(sm64_t[:], c_sm)
            ident_t = wp.tile([P, P], BF16)
            nc.sync.dma_start(ident_t[:], c_id)
            ones1_t = wp.tile([1, P], BF16)
            nc.sync.dma_start(ones1_t[:], c_ones)
            zrow = wp.tile([1, E], F32)
            nc.vector.memset(zrow[:], 0.0)
            z16 = wp.tile([P, T], BF16)
            nc.vector.memset(z16[:], 0.0)
            cntF_t = wp.tile([1, D * NGF], I32)
            nc.sync.dma_start(cntF_t[:], cntF_in)
            cntB_t = wp.tile([1, D * NGB], I32)
            nc.sync.dma_start(cntB_t[:], cntB_in)

            csum32 = [dr.tile([T + 1, E], F32, tag=f"csum32_{d}", name=f"csum32_{d}")
                      for d in range(D)]
            xbdr = [dr.tile([T, E], BF16, tag=f"xbdr_{d}", name=f"xbdr_{d}")
                    for d in range(D)]
            regF = nc.gpsimd.alloc_register()
            regB = nc.gpsimd.alloc_register()

            # ---------------- per-doc pipeline ----------------
            for d in range(D):
                with tc.tile_pool(name=f"xc{d}", bufs=1) as xc:
                    R_tok = xc.tile([P, NB, RP], BF16)
                    S_tok = xc.tile([P, NB, RP], BF16)

                    with tc.tile_pool(name=f"ab{d}", bufs=1) as ab, \
                         tc.tile_pool(name=f"xt{d}", bufs=2) as xtp, \
                         tc.tile_pool(name=f"xt1{d}", bufs=1) as xt1, \
                         tc.tile_pool(name=f"cs_sb{d}", bufs=2) as csp:

                        wbuf = ab.tile([P, NB // 2, E], F32R)
                        a_sb = ab.tile([P, NB], F32)
                        a_bc = ab.tile([P, T], BF16)
                        PaT = ab.tile([P, T], BF16)
                        PbT = ab.tile([P, T], BF16)
                        PcT = ab.tile([P, T], BF16)
                        PaTb = ab.tile([H2, T], BF16)
                        PbTb = ab.tile([H2, T], BF16)
                        PcTb = ab.tile([H2, T], BF16)
                        QiT = ab.tile([P, T], BF16)
                        QiTb = ab.tile([H2, T], BF16)
                        TOT2 = ab.tile([64, E], F32)
                        hilo = ab.tile([64, E], BF16)
                        tmpf = ab.tile([64, E], F32)

                        with tc.tile_pool(name=f"psW{d}", bufs=2, space="PSUM") as psW, \
                             tc.tile_pool(name=f"psT{d}", bufs=1, space="PSUM") as psT, \
                             tc.tile_pool(name=f"psC{d}", bufs=1, space="PSUM") as psC:
                            TOT_ps = psT.tile([64, E], F32, tag="totps")
                            nc.sync.dma_start(csum32[d][0:1, :], zrow[:])

                            for r in range(2):
                                for j in (2 * r, 2 * r + 1):
                                    tsl = slice(512 * j, 512 * (j + 1))
                                    xload = xtp.tile([P, 4, E], F32, tag="xload")
                                    nc.sync.dma_start(
                                        xload[:],
                                        xs[d][tsl, :].rearrange("(r p) e -> p r e", p=P))
                                    xb16s = xt1.tile([P, 4, E], BF16, tag="xb16s")
                                    nc.scalar.activation(xb16s[:], xload[:], AF.Identity)
                                    nc.sync.dma_start(
                                        xbdr[d][tsl, :].rearrange("(r p) e -> p r e", p=P),
                                        xb16s[:])
                                    xt = xt1.tile([P, EC, 512], BF16, tag="xt")
                                    for k in range(EC):
                                        nc.sync.dma_start(
                                            xt[:, k, :],
                                            xbdr[d][tsl, P * k:P * (k + 1)],
                                            transpose=True)
                                    # attention L1
                                    a1 = psW.tile([P, 512], F32, tag="apsum")
                                    a1b = psW.tile([H2, 512], F32, tag="apsumb")
                                    for k in range(EC):
                                        nc.tensor.matmul(a1[:], Wa1_16[:, k, 0:P],
                                                         xt[:, k, :], start=(k == 0),
                                                         stop=(k == EC - 1))
                                    for k in range(EC):
                                        nc.tensor.matmul(a1b[:], Wa1_16[:, k, P:HID],
                                                         xt[:, k, :], start=(k == 0),
                                                         stop=(k == EC - 1))
                                    h1a = xtp.tile([P, 512], BF16, tag="h1a")
                                    h1b = xtp.tile([H2, 512], BF16, tag="h1b")
                                    nc.scalar.activation(h1a[:], a1[:], AF.Relu,
                                                         bias=bias_t["ba1"][0][:])
                                    nc.scalar.activation(h1b[:], a1b[:], AF.Relu,
                                                         bias=bias_t["ba1"][1][:])
                                    # L2
                                    a2 = psW.tile([P, 512], F32, tag="apsum")
                                    a2b = psW.tile([H2, 512], F32, tag="apsumb")
                                    nc.tensor.matmul(a2[:], Wa2a[:, 0:P], h1a[:],
                                                     start=True, stop=False)
                                    nc.tensor.matmul(a2[:], Wa2b[:, 0:P], h1b[:],
                                                     start=False, stop=True)
                                    nc.tensor.matmul(a2b[:], Wa2a[:, P:HID], h1a[:],
                                                     start=True, stop=False)
                                    nc.tensor.matmul(a2b[:], Wa2b[:, P:HID], h1b[:],
                                                     start=False, stop=True)
                                    h2a = xtp.tile([P, 512], BF16, tag="h2a")
                                    h2b = xtp.tile([H2, 512], BF16, tag="h2b")
                                    nc.scalar.activation(h2a[:], a2[:], AF.Relu,
                                                         bias=bias_t["ba2"][0][:])
                                    nc.scalar.activation(h2b[:], a2b[:], AF.Relu,
                                                         bias=bias_t["ba2"][1][:])
                                    # L3: token-major a and row a
                                    aps_f = psW.tile([P, 512], F32, tag="apsum", name="apsf")
                                    aps = aps_f[:, 0:4]
                                    for jj in range(4):
                                        nc.tensor.matmul(aps[:, jj:jj + 1],
                                                         h2a[:, P * jj:P * (jj + 1)],
                                                         Wa3a[:], start=True, stop=False)
                                        nc.tensor.matmul(aps[:, jj:jj + 1],
                                                         h2b[:, P * jj:P * (jj + 1)],
                                                         Wa3b[:], start=False, stop=True)
                                    nc.vector.tensor_scalar(a_sb[:, 4 * j:4 * (j + 1)],
                                                            aps[:], ba3b_t[:], None, OP.add)
                                    arow = psW.tile([1, 512], F32, tag="apsumb", name="arow")
                                    nc.tensor.matmul(arow[:], Wa3a[:], h2a[:],
                                                     start=True, stop=False)
                                    nc.tensor.matmul(arow[:], Wa3b[:], h2b[:],
                                                     start=False, stop=True)
                                    arow16 = xtp.tile([1, 512], BF16, tag="arow16")
                                    nc.scalar.activation(arow16[:], arow[:], AF.Identity,
                                                         bias=ba3b_t[0:1, :])
                                    abc = psW.tile([P, 512], F32, tag="apsum", name="abc")
                                    nc.tensor.matmul(abc[:], ones1_t[:], arow16[:],
                                                     start=True, stop=True)
                                    nc.vector.tensor_copy(a_bc[:, tsl], abc[:])
                                    # projections Pa/Pb/Pc (feature-major bf16)
                                    for pi, (Ta, Tb) in enumerate(
                                            [(PaT, PaTb), (PbT, PbTb), (PcT, PcTb)]):
                                        pp = psW.tile([P, 512], F32, tag="apsum",
                                                      name=f"pp{pi}")
                                        ppb = psW.tile([H2, 512], F32, tag="apsumb",
                                                       name=f"ppb{pi}")
                                        for k in range(EC):
                                            nc.tensor.matmul(
                                                pp[:], W1_16[:, EC * pi + k, 0:P],
                                                xt[:, k, :], start=(k == 0),
                                                stop=(k == EC - 1))
                                        for k in range(EC):
                                            nc.tensor.matmul(
                                                ppb[:], W1_16[:, EC * pi + k, P:HID],
                                                xt[:, k, :], start=(k == 0),
                                                stop=(k == EC - 1))
                                        if pi % 2 == 0:
                                            nc.vector.tensor_copy(Ta[:, tsl], pp[:])
                                            nc.scalar.activation(Tb[:, tsl], ppb[:],
                                                                 AF.Identity)
                                        else:
                                            nc.scalar.activation(Ta[:, tsl], pp[:],
                                                                 AF.Identity)
                                            nc.vector.tensor_copy(Tb[:, tsl], ppb[:])
                                    # w = a*x (f32r) + block totals
                                    for bb in range(4 * j, 4 * (j + 1)):
                                        lbb = bb - 8 * r
                                        nc.vector.tensor_scalar_mul(
                                            wbuf[:, lbb, :], xload[:, bb - 4 * j, :],
                                            a_sb[:, bb:bb + 1])
                                        for h in range(2):
                                            nc.tensor.matmul(
                                                TOT_ps[:, 512 * h:512 * (h + 1)],
                                                oh64_t[:, 64 * bb:64 * (bb + 1)],
                                                wbuf[:, lbb, 512 * h:512 * (h + 1)],
                                                start=(bb == 0), stop=(bb % 8 == 7),
                                                skip_group_check=True)

                                # block offsets hi/lo
                                nc.vector.tensor_copy(TOT2[:], TOT_ps[:])
                                nc.scalar.activation(hilo[:], TOT2[:], AF.Identity)
                                nc.vector.tensor_copy(tmpf[32:48, :], hilo[32:48, :])
                                nc.vector.tensor_tensor(tmpf[32:48, :], TOT2[32:48, :],
                                                        tmpf[32:48, :], op=OP.subtract)
                                nc.vector.tensor_copy(hilo[32:48, :], tmpf[32:48, :])

                                # pass B: cumsum
                                for bb in range(8 * r, 8 * r + 8):
                                    lbb = bb - 8 * r
                                    cps = psC.tile([P, E], F32, tag="cps")
                                    for h in range(2):
                                        only = (bb == 0)
                                        nc.tensor.matmul(
                                            cps[:, 512 * h:512 * (h + 1)], U128_t[:],
                                            wbuf[:, lbb, 512 * h:512 * (h + 1)],
                                            start=True, stop=only)
                                        if not only:
                                            nc.tensor.matmul(
                                                cps[:, 512 * h:512 * (h + 1)],
                                                sm64_t[:, P * bb:P * (bb + 1)],
                                                hilo[:, 512 * h:512 * (h + 1)],
                                                start=False, stop=True)
                                    csb = csp.tile([P, E], F32, tag="csb")
                                    nc.vector.tensor_copy(csb[:, 0:512], cps[:, 0:512])
                                    nc.scalar.activation(csb[:, 512:E], cps[:, 512:E],
                                                         AF.Identity)
                                    nc.sync.dma_start(
                                        csum32[d][1 + P * bb:1 + P * (bb + 1), :], csb[:])

                        # ---- scan & R/S tables ----
                        with tc.tile_pool(name=f"psTr{d}", bufs=2, space="PSUM") as psTr:
                            nc.vector.tensor_tensor(PcT[:], PcT[:], a_bc[:], op=OP.mult)
                            nc.vector.tensor_tensor(PcTb[:], PcTb[:], a_bc[0:H2, :],
                                                    op=OP.mult)
                            nc.vector.tensor_tensor_scan(QiT[:], PcT[:], z16[:], 0.0,
                                                         OP.add, OP.add)
                            nc.vector.tensor_tensor_scan(QiTb[:], PcTb[:], z16[0:H2, :],
                                                         0.0, OP.add, OP.add)
                            # R = Pa - Q_excl ; S = Pb + Q_incl  (in place)
                            nc.vector.tensor_tensor(PaT[:, 1:T], PaT[:, 1:T],
                                                    QiT[:, 0:T - 1], op=OP.subtract)
                            nc.vector.tensor_tensor(PaTb[:, 1:T], PaTb[:, 1:T],
                                                    QiTb[:, 0:T - 1], op=OP.subtract)
                            nc.vector.tensor_tensor(PbT[:], PbT[:], QiT[:], op=OP.add)
                            nc.vector.tensor_tensor(PbTb[:], PbTb[:], QiTb[:], op=OP.add)
                            # transpose to token-major [t, 150] (pad RP)
                            for src_a, src_b, dst in [(PaT, PaTb, R_tok),
                                                      (PbT, PbTb, S_tok)]:
                                for bb in range(NB):
                                    tp = psTr.tile([P, RP], BF16, tag="tp")
                                    nc.tensor.transpose(
                                        tp[:, 0:P], src_a[:, P * bb:P * (bb + 1)],
                                        ident_t[:])
                                    nc.tensor.transpose(
                                        tp[:, P:HID], src_b[:, P * bb:P * (bb + 1)],
                                        ident_t[0:H2, 0:H2])
                                    nc.vector.tensor_copy(dst[:, bb, 0:HID],
                                                          tp[:, 0:HID])

                    # ---- pass C: span groups ----
                    with tc.tile_pool(name=f"span{d}", bufs=2) as sp, \
                         tc.tile_pool(name=f"asmp{d}", bufs=2) as asmp, \
                         tc.tile_pool(name=f"span1{d}", bufs=2) as sp1, \
                         tc.tile_pool(name=f"ix{d}", bufs=1) as ixp, \
                         tc.tile_pool(name=f"ps_s{d}", bufs=2, space="PSUM") as psS, \
                         tc.tile_pool(name=f"ps_sc{d}", bufs=1, space="PSUM") as psSc:

                        ix_t = {}
                        for nm in ["gsF", "geF", "c1F", "row", "gsB", "geB"]:
                            t = ixp.tile([P, SPAD // 16], I16, tag=f"ix{nm}")
                            nc.sync.dma_start(t[:], ix_in[(nm, d)])
                            ix_t[nm] = t
                        scps = psSc.tile([P, SPAD // P], F32, tag="scps")

                        # f32 span_emb path (groups of GSF)
                        for g in range(NGF):
                            gsl = slice((GSF // 16) * g, (GSF // 16) * (g + 1))
                            nc.gpsimd.load(regF, cntF_t[0:1, d * NGF + g:d * NGF + g + 1])
                            gs32 = sp.tile([P, GSF // P, E], F32, tag="gs32")
                            ge32 = sp.tile([P, GSF // P, E], F32, tag="ge32")
                            c0g = sp.tile([P, GSF // P, E], F32, tag="c0g")
                            c1g = sp.tile([P, GSF // P, E], F32, tag="c1g")
                            nc.gpsimd.dma_gather(gs32[:], xs[d], ix_t["gsF"][:, gsl],
                                                 GSF, regF, E)
                            nc.gpsimd.dma_gather(ge32[:], xs[d], ix_t["geF"][:, gsl],
                                                 GSF, regF, E)
                            nc.gpsimd.dma_gather(c0g[:], csum32[d][:], ix_t["gsF"][:, gsl],
                                                 GSF, regF, E)
                            nc.gpsimd.dma_gather(c1g[:], csum32[d][:], ix_t["c1F"][:, gsl],
                                                 GSF, regF, E)
                            asm = asmp.tile([P, GSF // P, GI], F32, tag="asm")
                            nc.scalar.activation(asm[:, :, 0:E], gs32[:], AF.Identity)
                            nc.vector.tensor_copy(asm[:, :, E:2 * E], ge32[:])
                            nc.vector.tensor_tensor(asm[:, :, 2 * E:GI], c1g[:], c0g[:],
                                                    op=OP.subtract)
                            nc.gpsimd.dma_scatter_add(se_out[d][:, :], asm[:],
                                                      ix_t["row"][:, gsl], GSF, regF, GI)

                        # bf16 scores path (groups of GSB)
                        for g in range(NGB):
                            gsl = slice((GSB // 16) * g, (GSB // 16) * (g + 1))
                            nc.gpsimd.load(regB, cntB_t[0:1, d * NGB + g:d * NGB + g + 1])
                            rg = sp1.tile([P, RP // P, GSB], BF16, tag="rg")
                            sg = sp1.tile([P, RP // P, GSB], BF16, tag="sg")
                            nc.gpsimd.dma_gather(
                                rg[:], R_tok[:], ix_t["gsB"][:, gsl], GSB, regB, RP,
                                transpose=True, sbuf_tokens_per_rank=P,
                                sbuf_free_dim_per_rank=2 * RP)
                            nc.gpsimd.dma_gather(
                                sg[:], S_tok[:], ix_t["geB"][:, gsl], GSB, regB, RP,
                                transpose=True, sbuf_tokens_per_rank=P,
                                sbuf_free_dim_per_rank=2 * RP)
                            h1p = sp1.tile([P, RP // P, GSB], BF16, tag="h1p")
                            nc.vector.tensor_tensor(h1p[:], rg[:], sg[:], op=OP.add)
                            s1a = sp1.tile([P, GSB], BF16, tag="s1a")
                            s1b = sp1.tile([H2, GSB], BF16, tag="s1b")
                            nc.scalar.activation(s1a[:], h1p[:, 0, :], AF.Relu,
                                                 bias=bias_t["bs1"][0][:])
                            nc.scalar.activation(s1b[:], h1p[0:H2, 1, :], AF.Relu,
                                                 bias=bias_t["bs1"][1][:])
                            h2 = psS.tile([P, GSB], F32, tag="h2")
                            h2b = psS.tile([H2, GSB], F32, tag="h2b")
                            nc.tensor.matmul(h2[:], Ws2a[:, 0:P], s1a[:],
                                             start=True, stop=False)
                            nc.tensor.matmul(h2[:], Ws2b[:, 0:P], s1b[:],
                                             start=False, stop=True)
                            nc.tensor.matmul(h2b[:], Ws2a[:, P:HID], s1a[:],
                                             start=True, stop=False)
                            nc.tensor.matmul(h2b[:], Ws2b[:, P:HID], s1b[:],
                                             start=False, stop=True)
                            s2a = sp1.tile([P, GSB], BF16, tag="s2a")
                            s2b = sp1.tile([H2, GSB], BF16, tag="s2b")
                            nc.scalar.activation(s2a[:], h2[:], AF.Relu,
                                                 bias=bias_t["bs2"][0][:])
                            nc.scalar.activation(s2b[:], h2b[:], AF.Relu,
                                                 bias=bias_t["bs2"][1][:])
                            for c in range(GSB // P):
                                col = g * (GSB // P) + c
                                nc.tensor.matmul(scps[:, col:col + 1],
                                                 s2a[:, P * c:P * (c + 1)], Ws3a[:],
                                                 start=True, stop=False)
                                nc.tensor.matmul(scps[:, col:col + 1],
                                                 s2b[:, P * c:P * (c + 1)], Ws3b[:],
                                                 start=False, stop=True)

                        scmask_t = ixp.tile([P, SPAD // P], I32, tag="scmask")
                        nc.sync.dma_start(scmask_t[:], scmask_in[d])
                        sc_f = ixp.tile([P, SPAD // P], F32, tag="scf")
                        sc_z = ixp.tile([P, SPAD // P], F32, tag="scz")
                        sc_m = ixp.tile([P, SPAD // P], F32, tag="scm2")
                        nc.vector.memset(sc_z[:], 0.0)
                        nc.vector.tensor_scalar(sc_f[:], scps[:], bs3b_t[:], None, OP.add)
                        nc.vector.select(sc_m[:], scmask_t[:], sc_f[:], sc_z[:])
                        nc.sync.dma_start(sc_out[d][:], sc_m[:])

    nc.compile()
    return nc


def _get_program():
    if "nc" not in _PROGRAM_CACHE:
        _PROGRAM_CACHE["nc"] = _build_program()
    return _PROGRAM_CACHE["nc"]


def _host_prep(inputs):
    x = np.ascontiguousarray(np.asarray(inputs["batch_embeds"], dtype=np.float32))
    starts = np.asarray(inputs["starts"]).astype(np.int64)
    lengths = np.asarray(inputs["lengths"]).astype(np.int64)
    num_spans = np.asarray(inputs["num_spans"]).astype(np.int64)
    ends = starts + lengths

    wmap = {}
    for nm in ["Wa1", "Wa2", "Wa3", "Ws1", "Ws2", "Ws3"]:
        wmap[nm] = np.ascontiguousarray(np.asarray(inputs[nm], dtype=np.float32))
    for nm in ["ba1", "ba2", "bs1", "bs2"]:
        wmap[nm] = np.asarray(inputs[nm], dtype=np.float32).reshape(HID, 1)
    wmap["ba3b"] = np.broadcast_to(
        np.asarray(inputs["ba3"], np.float32).reshape(1, 1), (P, 1)).copy()
    wmap["bs3b"] = np.broadcast_to(
        np.asarray(inputs["bs3"], np.float32).reshape(1, 1), (P, 1)).copy()

    U = np.triu(np.ones((P, P), np.float32))
    onehot = np.zeros((P, NB * 64), np.float32)
    for bb in range(NB):
        onehot[:, 64 * bb + bb] = 1.0
        onehot[:, 64 * bb + 32 + bb] = 1.0
    stepmask = np.zeros((64, NB * P), np.float32)
    for bb in range(NB):
        for k in range(64):
            if (k < 16 and k < bb) or (32 <= k < 48 and k - 32 < bb):
                stepmask[k, P * bb:P * (bb + 1)] = 1.0
    ident = np.eye(P, dtype=np.float32)

    def wrap_doc(vals, ns, group, dummy):
        a = np.full(SPAD, -1, np.int64)
        n = min(ns, S)
        a[:n] = vals[:n]
        for g0 in range(0, SPAD, group):
            if ns <= g0:
                a[g0] = dummy
        return _wrap_idx(a)

    in_maps = []
    for c in range(NCORES):
        m = dict(wmap)
        m["U128"] = U
        m["onehot64"] = onehot
        m["stepmask64"] = stepmask.astype(ml_dtypes.bfloat16)
        m["ident"] = ident.astype(ml_dtypes.bfloat16)
        m["ones1"] = np.ones((1, P), ml_dtypes.bfloat16)
        cntsF = np.zeros((1, D * NGF), np.int32)
        cntsB = np.zeros((1, D * NGB), np.int32)
        for d in range(D):
            doc = c * D + d
            ns = int(num_spans[doc])
            m[f"x{d}"] = x[doc]
            st, en = starts[doc], ends[doc]
            rows = np.arange(S)
            m[f"ix_gsF{d}"] = wrap_doc(st, ns, GSF, 0)
            m[f"ix_geF{d}"] = wrap_doc(en, ns, GSF, 0)
            m[f"ix_c1F{d}"] = wrap_doc(en + 1, ns, GSF, 0)
            m[f"ix_row{d}"] = wrap_doc(rows, ns, GSF, SPAD)
            m[f"ix_gsB{d}"] = wrap_doc(st, ns, GSB, 0)
            m[f"ix_geB{d}"] = wrap_doc(en, ns, GSB, 0)
            for g in range(NGF):
                cntsF[0, d * NGF + g] = max(1, min(ns - GSF * g, GSF))
            for g in range(NGB):
                cntsB[0, d * NGB + g] = max(1, min(ns - GSB * g, GSB))
            scm = np.zeros(SPAD, np.int32)
            scm[:ns] = 1
            m[f"scmask{d}"] = scm.reshape(SPAD // P, P).T.copy()
        m["cntF"] = cntsF
        m["cntB"] = cntsB
        in_maps.append(m)
    return in_maps


LAST_RESULTS = None


def kernel(**inputs):
    global LAST_RESULTS
    nc = _get_program()
    in_maps = _host_prep(inputs)
    res = bass_utils.run_bass_kernel_spmd(nc, in_maps, core_ids=list(range(NCORES)))
    LAST_RESULTS = res
    span_emb = np.empty((B, S, GI), np.float32)
    scores = np.empty((B, S, 1), np.float32)
    for c in range(NCORES):
        r = res.results[c]
        for d in range(D):
            doc = c * D + d
            span_emb[doc] = r[f"se{d}"][:S, :]
            sc = r[f"sc{d}"].T.reshape(SPAD)[:S]
            scores[doc] = sc[:, None]
    return span_emb, scores


if __name__ == "__main__":
    import time
    ref = np.load("/root/problem/work/ref_cache.npz")
    inputs = {k: ref[k] for k in ref.files if k not in ("span_emb", "mention_scores")}
    t0 = time.time()
    se, ms = kernel(**inputs)
    print("run time", time.time() - t0)
    for nm, got, want in [("span_emb", se, ref["span_emb"]),
                          ("scores", ms, ref["mention_scores"])]:
        rel = np.linalg.norm((got - want).ravel()) / np.linalg.norm(want.ravel())
        print(f"{nm}: rel={rel:.3e} maxabs={np.abs(got - want).max():.3e}")
